# revision 6
# baseline (speedup 1.0000x reference)
"""CrossModalAttention Trainium2 kernel.

Per sample (C-major layouts, n = H*W = 1024 tokens):
    cnn_proj = W_cnn @ xc + b_cnn                [512, n]  f32 (residual path, exact)
    q  = (W_q @ W_cnn) @ xc + (W_q@b_cnn + b_q)  [512, n]  host-fused projection
    eff_proj = W_eff @ xe + b_eff                [512, n]
    k  = W_k @ eff_proj + b_k                    [512, n]
    vT = eff_proj^T @ (gamma*W_v)^T + gamma*b_v  [n, 512]  (v transposed, gamma folded)
    ST = k^T @ q                                 [n(keys m), n(queries)]
    ET = exp(ST)            (logits well-conditioned; no max-subtraction needed)
    s[n] = sum_m ET[m, n]   (ones-vector matmul accumulated in PSUM)
    out = (vT^T @ ET) * (1/s  broadcast) + cnn_proj

Sharding: data-parallel over batch, 4 samples per core on 8 cores.
Weight transposes / fusions are host-side; no on-chip transposes.
Projection GEMMs from DRAM inputs run as float32r (full PE rate);
the gamma-scaled attention path runs in bf16.
"""

import numpy as np
import ml_dtypes

import concourse.bacc as bacc
import concourse.bass as bass
import concourse.tile as tile
from concourse import mybir
from concourse.bass import ts
from concourse.bass_utils import run_bass_kernel_spmd

F32 = mybir.dt.float32
F32R = mybir.dt.float32r
BF16 = mybir.dt.bfloat16
AF = mybir.ActivationFunctionType

B, HW_N = 32, 1024
C_CNN, C_EFF, HID = 512, 1280, 512
N_CORES = 8
S = B // N_CORES          # samples per core
KC_CNN = C_CNN // 128     # 4
KC_EFF = C_EFF // 128     # 10
MT = HID // 128           # 4 output-channel tiles
NT = HW_N // 128          # 8 token tiles
NCH = HW_N // 512         # 2 free-dim chunks of 512


def build_program():
    nc = bacc.Bacc("TRN2", target_bir_lowering=False, debug=False,
                   num_devices=N_CORES)

    xc_d = nc.dram_tensor("xc", [S, KC_CNN, 128, HW_N], F32R, kind="ExternalInput")
    xe_d = nc.dram_tensor("xe", [S, KC_EFF, 128, HW_N], BF16, kind="ExternalInput")
    wct_d = nc.dram_tensor("wct", [KC_CNN, 128, HID], F32R, kind="ExternalInput")
    wqct_d = nc.dram_tensor("wqct", [KC_CNN, 128, HID], F32R, kind="ExternalInput")
    wet_d = nc.dram_tensor("wet", [KC_EFF, 128, HID], BF16, kind="ExternalInput")
    wkt_d = nc.dram_tensor("wkt", [MT, 128, HID], BF16, kind="ExternalInput")
    wvt_d = nc.dram_tensor("wvt", [MT, 128, HID], BF16, kind="ExternalInput")
    bcnn_d = nc.dram_tensor("bcnn", [MT, 128, 1], F32, kind="ExternalInput")
    bqc_d = nc.dram_tensor("bqc", [MT, 128, 1], F32, kind="ExternalInput")
    beff_d = nc.dram_tensor("beff", [MT, 128, 1], F32, kind="ExternalInput")
    bk_d = nc.dram_tensor("bk", [MT, 128, 1], F32, kind="ExternalInput")
    bv_d = nc.dram_tensor("bv", [1, HID], F32, kind="ExternalInput")
    out_d = nc.dram_tensor("out", [S, MT, 128, HW_N], F32, kind="ExternalOutput")

    with tile.TileContext(nc) as tc:
        with (
            tc.tile_pool(name="singles", bufs=1) as singles,
            tc.tile_pool(name="xs", bufs=6) as xs_pool,
            tc.tile_pool(name="proj", bufs=2) as proj_pool,
            tc.tile_pool(name="attn", bufs=1) as attn_pool,
            tc.tile_pool(name="small", bufs=2) as small_pool,
            tc.tile_pool(name="outp", bufs=2) as out_pool,
            tc.tile_pool(name="mmps", bufs=4, space="PSUM") as mmps,
            tc.tile_pool(name="sums", bufs=1, space="PSUM") as sums_pool,
            tc.tile_pool(name="bps", bufs=1, space="PSUM") as bps_pool,
        ):
            # ---- constants / weights (resident for the whole kernel) ----
            wcnn = singles.tile([128, KC_CNN, HID], F32R, tag="wcnn")
            wqc = singles.tile([128, KC_CNN, HID], F32R, tag="wqc")
            weff = singles.tile([128, KC_EFF, HID], BF16, tag="weff")
            wk = singles.tile([128, MT, HID], BF16, tag="wk")
            wv = singles.tile([128, MT, HID], BF16, tag="wv")
            for ko in range(KC_CNN):
                nc.sync.dma_start(out=wcnn[:, ko, :], in_=wct_d[ko])
            for w_sb, w_dr, kn in ((wqc, wqct_d, KC_CNN),
                                   (weff, wet_d, KC_EFF), (wk, wkt_d, MT),
                                   (wv, wvt_d, MT)):
                for ko in range(kn):
                    nc.gpsimd.dma_start(out=w_sb[:, ko, :], in_=w_dr[ko])

            bcnn = singles.tile([128, MT], F32, tag="bcnn")
            bqc = singles.tile([128, MT], F32, tag="bqc")
            beff = singles.tile([128, MT], F32, tag="beff")
            bk = singles.tile([128, MT], F32, tag="bk")
            for b_sb, b_dr in ((bcnn, bcnn_d), (bqc, bqc_d), (beff, beff_d),
                               (bk, bk_d)):
                for ko in range(MT):
                    nc.gpsimd.dma_start(out=b_sb[:, ko:ko + 1], in_=b_dr[ko])

            bvt = singles.tile([128, HID], F32, tag="bvt")
            nc.gpsimd.dma_start(out=bvt[:], in_=bv_d[0].partition_broadcast(128))
            ones_bf = singles.tile([128, 1], BF16, tag="ones_bf")
            nc.vector.memset(ones_bf[:], 1.0)
            ones_row = singles.tile([1, 128], BF16, tag="ones_row")
            nc.vector.memset(ones_row[:], 1.0)

            def proj_sweep(dst, w_sb, bias, rhs_tiles, kcs, out_dtype_note=None):
                """dst[:, m, ch*512:] = sum_kc w_sb[:,kc,m*128:].T @ rhs(kc,ch) + bias[m]"""
                for ch in range(NCH):
                    ps = [mmps.tile([128, 512], F32, tag="mmps", name="ps")
                          for _ in range(MT)]
                    for kc in range(kcs):
                        rhs = rhs_tiles(kc, ch)
                        for m in range(MT):
                            nc.tensor.matmul(
                                ps[m][:], w_sb[:, kc, ts(m, 128)], rhs,
                                start=(kc == 0), stop=(kc == kcs - 1))
                    for m in range(MT):
                        nc.scalar.activation(
                            out=dst[:, m, ts(ch, 512)], in_=ps[m][:],
                            func=AF.Identity, bias=bias[:, m:m + 1], scale=1.0)

            for s in range(S):
                # ---- A: cnn_proj (f32, residual) ----
                cnn_proj = proj_pool.tile([128, MT, HW_N], F32, tag="cnn_proj")

                def rhs_xc(kc, ch, s=s):
                    xt = xs_pool.tile([128, 512], F32R, tag="xt", name="xt")
                    nc.sync.dma_start(out=xt[:], in_=xc_d[s, kc][:, ts(ch, 512)])
                    return xt[:]

                proj_sweep(cnn_proj, wcnn, bcnn, rhs_xc, KC_CNN)

                # ---- Q: q = Wqc @ xc + bqc (bf16, second xc sweep) ----
                q_sb = proj_pool.tile([128, MT, HW_N], BF16, tag="q")
                proj_sweep(q_sb, wqc, bqc, rhs_xc, KC_CNN)

                # ---- B: eff_proj (bf16) ----
                eff_proj = proj_pool.tile([128, MT, HW_N], BF16, tag="eff_proj")

                def rhs_xe(kc, ch, s=s):
                    xt = xs_pool.tile([128, 512], BF16, tag="xt", name="xt")
                    nc.sync.dma_start(out=xt[:], in_=xe_d[s, kc][:, ts(ch, 512)])
                    return xt[:]

                proj_sweep(eff_proj, weff, beff, rhs_xe, KC_EFF)

                # ---- D: k = W_k @ eff_proj + b_k (bf16) ----
                k_sb = proj_pool.tile([128, MT, HW_N], BF16, tag="k")
                proj_sweep(k_sb, wk, bk,
                           lambda kc, ch: eff_proj[:, kc, ts(ch, 512)], MT)

                # ---- E: vT = eff_proj^T @ wv + bv (bf16) ----
                vT = attn_pool.tile([128, NT, HID], BF16, tag="vT")
                for mt in range(NT):
                    ps_e = mmps.tile([128, 512], F32, tag="mmps")
                    for kc in range(MT):
                        nc.tensor.matmul(
                            ps_e[:], eff_proj[:, kc, ts(mt, 128)], wv[:, kc, :],
                            start=(kc == 0), stop=(kc == MT - 1))
                    nc.vector.tensor_add(out=vT[:, mt, :], in0=ps_e[:],
                                         in1=bvt[:])

                # ---- F: ST = k^T @ q; ET = exp(ST); sums += 1^T ET ----
                eT = attn_pool.tile([128, NT, HW_N], BF16, tag="eT")
                sums_ps = sums_pool.tile([1, HW_N], F32, tag="sums")
                def emit_sums(mt):
                    for ch in range(NCH):
                        nc.tensor.matmul(
                            sums_ps[:, ts(ch, 512)], ones_bf[:],
                            eT[:, mt, ts(ch, 512)],
                            start=(mt == 0), stop=(mt == NT - 1))

                for mt in range(NT):
                    ps_f = [mmps.tile([128, 512], F32, tag="mmps", name="ps")
                            for _ in range(NCH)]
                    for kc in range(MT):
                        for ch in range(NCH):
                            nc.tensor.matmul(
                                ps_f[ch][:], k_sb[:, kc, ts(mt, 128)],
                                q_sb[:, kc, ts(ch, 512)],
                                start=(kc == 0), stop=(kc == MT - 1))
                    for ch in range(NCH):
                        nc.scalar.activation(
                            out=eT[:, mt, ts(ch, 512)], in_=ps_f[ch][:],
                            func=AF.Exp)
                    if mt > 0:
                        emit_sums(mt - 1)
                emit_sums(NT - 1)

                # ---- recip = 1 / sums (bf16; gamma already folded into wv) ----
                recip = small_pool.tile([1, HW_N], BF16, tag="recip")
                with nc.allow_low_precision(
                        reason="softmax denominators are gamma-scaled; "
                               "bf16 suffices"):
                    nc.vector.reciprocal(out=recip[:], in_=sums_ps[:])

                # ---- J: unnorm = vT^T @ ET; K: out = unnorm*bcast + cnn_proj ----
                bcast = small_pool.tile([128, HW_N], F32, tag="bcast")
                for ct in range(MT):
                    ps_j = [mmps.tile([128, 512], F32, tag="mmps", name="ps")
                            for _ in range(NCH)]
                    for mt in range(NT):
                        for ch in range(NCH):
                            nc.tensor.matmul(
                                ps_j[ch][:], vT[:, mt, ts(ct, 128)],
                                eT[:, mt, ts(ch, 512)],
                                start=(mt == 0), stop=(mt == NT - 1))
                    if ct == 0:
                        # broadcast 1/s across partitions via ones matmul
                        ps_b = bps_pool.tile([128, HW_N], F32, tag="bps")
                        for ch in range(NCH):
                            nc.tensor.matmul(
                                ps_b[:, ts(ch, 512)], ones_row[:],
                                recip[:, ts(ch, 512)],
                                start=True, stop=True)
                            nc.scalar.activation(
                                out=bcast[:, ts(ch, 512)],
                                in_=ps_b[:, ts(ch, 512)], func=AF.Copy)
                    out_t = out_pool.tile([128, HW_N], F32, tag="out_t")
                    for ch in range(NCH):
                        nc.vector.tensor_mul(
                            out=out_t[:, ts(ch, 512)], in0=ps_j[ch][:],
                            in1=bcast[:, ts(ch, 512)])
                        nc.vector.tensor_add(
                            out=out_t[:, ts(ch, 512)],
                            in0=out_t[:, ts(ch, 512)],
                            in1=cnn_proj[:, ct, ts(ch, 512)])
                    nc.sync.dma_start(out=out_d[s, ct], in_=out_t[:])

    nc.compile()
    return nc


_CACHED_NC = None


def _get_nc():
    global _CACHED_NC
    if _CACHED_NC is None:
        _CACHED_NC = build_program()
    return _CACHED_NC


def make_in_maps(cnn_features, efficient_features, W_cnn, b_cnn, W_eff, b_eff,
                 W_q, b_q, W_k, b_k, W_v, b_v, gamma):
    f = np.float32
    bf = ml_dtypes.bfloat16
    g = np.asarray(gamma, f).reshape(-1)[0]
    W_qc = (np.asarray(W_q, f) @ np.asarray(W_cnn, f)).astype(f)
    b_qc = (np.asarray(W_q, f) @ np.asarray(b_cnn, f) + np.asarray(b_q, f))
    shared = {
        "wct": np.ascontiguousarray(np.asarray(W_cnn, f).T).reshape(KC_CNN, 128, HID),
        "wqct": np.ascontiguousarray(W_qc.T).reshape(KC_CNN, 128, HID),
        "wet": np.ascontiguousarray(np.asarray(W_eff, f).T).astype(bf).reshape(KC_EFF, 128, HID),
        "wkt": np.ascontiguousarray(np.asarray(W_k, f).T).astype(bf).reshape(MT, 128, HID),
        "wvt": np.ascontiguousarray(np.asarray(W_v, f).T * g).astype(bf).reshape(MT, 128, HID),
        "bcnn": np.ascontiguousarray(b_cnn, f).reshape(MT, 128, 1),
        "bqc": np.ascontiguousarray(b_qc, f).reshape(MT, 128, 1),
        "beff": np.ascontiguousarray(b_eff, f).reshape(MT, 128, 1),
        "bk": np.ascontiguousarray(b_k, f).reshape(MT, 128, 1),
        "bv": (np.ascontiguousarray(b_v, f) * g).reshape(1, HID),
    }
    xc = np.ascontiguousarray(cnn_features, f).reshape(B, KC_CNN, 128, HW_N)
    xe = np.ascontiguousarray(efficient_features, f).astype(bf).reshape(B, KC_EFF, 128, HW_N)
    in_maps = []
    for c in range(N_CORES):
        m = dict(shared)
        m["xc"] = np.ascontiguousarray(xc[c * S:(c + 1) * S])
        m["xe"] = np.ascontiguousarray(xe[c * S:(c + 1) * S])
        in_maps.append(m)
    return in_maps


def kernel(**inputs) -> np.ndarray:
    inputs = {k: np.asarray(v) for k, v in inputs.items()}
    nc = _get_nc()
    in_maps = make_in_maps(**inputs)
    res = run_bass_kernel_spmd(nc, in_maps, list(range(N_CORES)))
    out = np.concatenate([res.results[c]["out"] for c in range(N_CORES)], axis=0)
    return out.reshape(B, HID, 32, 32)


# revision 7
# speedup vs baseline: 1.0299x; 1.0299x over previous
"""CrossModalAttention Trainium2 kernel.

Per sample (C-major layouts, n = H*W = 1024 tokens):
    cnn_proj = W_cnn @ xc + b_cnn                [512, n]  f32 (residual path, exact)
    q  = (W_q @ W_cnn) @ xc + (W_q@b_cnn + b_q)  [512, n]  host-fused projection
    eff_proj = W_eff @ xe + b_eff                [512, n]
    k  = W_k @ eff_proj + b_k                    [512, n]
    vT = eff_proj^T @ (gamma*W_v)^T + gamma*b_v  [n, 512]  (v transposed, gamma folded)
    ST = k^T @ q                                 [n(keys m), n(queries)]
    ET = exp(ST)            (logits well-conditioned; no max-subtraction needed)
    s[n] = sum_m ET[m, n]   (ones-vector matmul accumulated in PSUM)
    out = (vT^T @ ET) * (1/s  broadcast) + cnn_proj

Sharding: data-parallel over batch, 4 samples per core on 8 cores.
Weight transposes / fusions are host-side; no on-chip transposes.
Projection GEMMs from DRAM inputs run as float32r (full PE rate);
the gamma-scaled attention path runs in bf16.
"""

import numpy as np
import ml_dtypes

import concourse.bacc as bacc
import concourse.bass as bass
import concourse.tile as tile
from concourse import mybir
from concourse.bass import ts
from concourse.bass_utils import run_bass_kernel_spmd

F32 = mybir.dt.float32
F32R = mybir.dt.float32r
BF16 = mybir.dt.bfloat16
AF = mybir.ActivationFunctionType

B, HW_N = 32, 1024
C_CNN, C_EFF, HID = 512, 1280, 512
N_CORES = 8
S = B // N_CORES          # samples per core
KC_CNN = C_CNN // 128     # 4
KC_EFF = C_EFF // 128     # 10
MT = HID // 128           # 4 output-channel tiles
NT = HW_N // 128          # 8 token tiles
NCH = HW_N // 512         # 2 free-dim chunks of 512


def build_program():
    nc = bacc.Bacc("TRN2", target_bir_lowering=False, debug=False,
                   num_devices=N_CORES)

    xc_d = nc.dram_tensor("xc", [S, KC_CNN, 128, HW_N], F32R, kind="ExternalInput")
    xe_d = nc.dram_tensor("xe", [S, KC_EFF, 128, HW_N], BF16, kind="ExternalInput")
    wct_d = nc.dram_tensor("wct", [KC_CNN, 128, HID], F32R, kind="ExternalInput")
    wqct_d = nc.dram_tensor("wqct", [KC_CNN, 128, HID], F32R, kind="ExternalInput")
    wet_d = nc.dram_tensor("wet", [KC_EFF, 128, HID], BF16, kind="ExternalInput")
    wkt_d = nc.dram_tensor("wkt", [MT, 128, HID], BF16, kind="ExternalInput")
    wvt_d = nc.dram_tensor("wvt", [MT, 128, HID], BF16, kind="ExternalInput")
    bcnn_d = nc.dram_tensor("bcnn", [MT, 128, 1], F32, kind="ExternalInput")
    bqc_d = nc.dram_tensor("bqc", [MT, 128, 1], F32, kind="ExternalInput")
    beff_d = nc.dram_tensor("beff", [MT, 128, 1], F32, kind="ExternalInput")
    bk_d = nc.dram_tensor("bk", [MT, 128, 1], F32, kind="ExternalInput")
    bv_d = nc.dram_tensor("bv", [1, HID], F32, kind="ExternalInput")
    out_d = nc.dram_tensor("out", [S, MT, 128, HW_N], F32, kind="ExternalOutput")

    with tile.TileContext(nc) as tc:
        with (
            tc.tile_pool(name="singles", bufs=1) as singles,
            tc.tile_pool(name="xs", bufs=6) as xs_pool,
            tc.tile_pool(name="proj", bufs=2) as proj_pool,
            tc.tile_pool(name="attn", bufs=1) as attn_pool,
            tc.tile_pool(name="small", bufs=2) as small_pool,
            tc.tile_pool(name="outp", bufs=2) as out_pool,
            tc.tile_pool(name="mmps", bufs=4, space="PSUM") as mmps,
            tc.tile_pool(name="sums", bufs=1, space="PSUM") as sums_pool,
            tc.tile_pool(name="bps", bufs=1, space="PSUM") as bps_pool,
        ):
            # ---- constants / weights (resident for the whole kernel) ----
            wcnn = singles.tile([128, KC_CNN, HID], F32R, tag="wcnn")
            wqc = singles.tile([128, KC_CNN, HID], F32R, tag="wqc")
            weff = singles.tile([128, KC_EFF, HID], BF16, tag="weff")
            wk = singles.tile([128, MT, HID], BF16, tag="wk")
            wv = singles.tile([128, MT, HID], BF16, tag="wv")
            for ko in range(KC_CNN):
                nc.sync.dma_start(out=wcnn[:, ko, :], in_=wct_d[ko])
            for ko in range(KC_CNN):
                nc.sync.dma_start(out=wqc[:, ko, :], in_=wqct_d[ko])
            for w_sb, w_dr, kn in ((weff, wet_d, KC_EFF), (wk, wkt_d, MT),
                                   (wv, wvt_d, MT)):
                for ko in range(kn):
                    nc.gpsimd.dma_start(out=w_sb[:, ko, :], in_=w_dr[ko])

            bcnn = singles.tile([128, MT], F32, tag="bcnn")
            bqc = singles.tile([128, MT], F32, tag="bqc")
            beff = singles.tile([128, MT], F32, tag="beff")
            bk = singles.tile([128, MT], F32, tag="bk")
            for b_sb, b_dr in ((bcnn, bcnn_d), (bqc, bqc_d), (beff, beff_d),
                               (bk, bk_d)):
                for ko in range(MT):
                    nc.gpsimd.dma_start(out=b_sb[:, ko:ko + 1], in_=b_dr[ko])

            bvt = singles.tile([128, HID], F32, tag="bvt")
            nc.gpsimd.dma_start(out=bvt[:], in_=bv_d[0].partition_broadcast(128))
            ones_bf = singles.tile([128, 1], BF16, tag="ones_bf")
            nc.vector.memset(ones_bf[:], 1.0)
            ones_row = singles.tile([1, 128], BF16, tag="ones_row")
            nc.vector.memset(ones_row[:], 1.0)

            def proj_sweep(dst, w_sb, bias, rhs_tiles, kcs, out_dtype_note=None):
                """dst[:, m, ch*512:] = sum_kc w_sb[:,kc,m*128:].T @ rhs(kc,ch) + bias[m]"""
                for ch in range(NCH):
                    ps = [mmps.tile([128, 512], F32, tag="mmps", name="ps")
                          for _ in range(MT)]
                    for kc in range(kcs):
                        rhs = rhs_tiles(kc, ch)
                        for m in range(MT):
                            nc.tensor.matmul(
                                ps[m][:], w_sb[:, kc, ts(m, 128)], rhs,
                                start=(kc == 0), stop=(kc == kcs - 1))
                    for m in range(MT):
                        nc.scalar.activation(
                            out=dst[:, m, ts(ch, 512)], in_=ps[m][:],
                            func=AF.Identity, bias=bias[:, m:m + 1], scale=1.0)

            for s in range(S):
                # ---- A: cnn_proj (f32, residual) ----
                cnn_proj = proj_pool.tile([128, MT, HW_N], F32, tag="cnn_proj")

                def rhs_xc(kc, ch, s=s):
                    xt = xs_pool.tile([128, 512], F32R, tag="xt", name="xt")
                    nc.sync.dma_start(out=xt[:], in_=xc_d[s, kc][:, ts(ch, 512)])
                    return xt[:]

                proj_sweep(cnn_proj, wcnn, bcnn, rhs_xc, KC_CNN)

                # ---- Q: q = Wqc @ xc + bqc (bf16, second xc sweep) ----
                q_sb = proj_pool.tile([128, MT, HW_N], BF16, tag="q", bufs=1)
                proj_sweep(q_sb, wqc, bqc, rhs_xc, KC_CNN)

                # ---- B: eff_proj (bf16) ----
                eff_proj = proj_pool.tile([128, MT, HW_N], BF16, tag="eff_proj", bufs=1)

                def rhs_xe(kc, ch, s=s):
                    xt = xs_pool.tile([128, 512], BF16, tag="xt", name="xt")
                    nc.sync.dma_start(out=xt[:], in_=xe_d[s, kc][:, ts(ch, 512)])
                    return xt[:]

                proj_sweep(eff_proj, weff, beff, rhs_xe, KC_EFF)

                # ---- D: k = W_k @ eff_proj + b_k (bf16) ----
                k_sb = proj_pool.tile([128, MT, HW_N], BF16, tag="k", bufs=1)
                proj_sweep(k_sb, wk, bk,
                           lambda kc, ch: eff_proj[:, kc, ts(ch, 512)], MT)

                # ---- E: vT = eff_proj^T @ wv + bv (bf16) ----
                vT = attn_pool.tile([128, NT, HID], BF16, tag="vT")
                for mt in range(NT):
                    ps_e = mmps.tile([128, 512], F32, tag="mmps")
                    for kc in range(MT):
                        nc.tensor.matmul(
                            ps_e[:], eff_proj[:, kc, ts(mt, 128)], wv[:, kc, :],
                            start=(kc == 0), stop=(kc == MT - 1))
                    nc.vector.tensor_add(out=vT[:, mt, :], in0=ps_e[:],
                                         in1=bvt[:])

                # ---- F: ST = k^T @ q; ET = exp(ST); sums += 1^T ET ----
                eT = attn_pool.tile([128, NT, HW_N], BF16, tag="eT")
                sums_ps = sums_pool.tile([1, HW_N], F32, tag="sums")
                def emit_sums(mt):
                    for ch in range(NCH):
                        nc.tensor.matmul(
                            sums_ps[:, ts(ch, 512)], ones_bf[:],
                            eT[:, mt, ts(ch, 512)],
                            start=(mt == 0), stop=(mt == NT - 1))

                for mt in range(NT):
                    ps_f = [mmps.tile([128, 512], F32, tag="mmps", name="ps")
                            for _ in range(NCH)]
                    for kc in range(MT):
                        for ch in range(NCH):
                            nc.tensor.matmul(
                                ps_f[ch][:], k_sb[:, kc, ts(mt, 128)],
                                q_sb[:, kc, ts(ch, 512)],
                                start=(kc == 0), stop=(kc == MT - 1))
                    for ch in range(NCH):
                        nc.scalar.activation(
                            out=eT[:, mt, ts(ch, 512)], in_=ps_f[ch][:],
                            func=AF.Exp)
                    if mt > 0:
                        emit_sums(mt - 1)
                emit_sums(NT - 1)

                # ---- recip = 1 / sums (bf16; gamma already folded into wv) ----
                recip = small_pool.tile([1, HW_N], BF16, tag="recip")
                with nc.allow_low_precision(
                        reason="softmax denominators are gamma-scaled; "
                               "bf16 suffices"):
                    nc.vector.reciprocal(out=recip[:], in_=sums_ps[:])

                # ---- J: unnorm = vT^T @ ET, drained to SBUF via ACT ----
                unnorm = attn_pool.tile([128, MT, HW_N], F32, tag="unnorm")
                for ct in range(MT):
                    ps_j = [mmps.tile([128, 512], F32, tag="mmps", name="ps")
                            for _ in range(NCH)]
                    for mt in range(NT):
                        for ch in range(NCH):
                            nc.tensor.matmul(
                                ps_j[ch][:], vT[:, mt, ts(ct, 128)],
                                eT[:, mt, ts(ch, 512)],
                                start=(mt == 0), stop=(mt == NT - 1))
                    for ch in range(NCH):
                        nc.scalar.activation(
                            out=unnorm[:, ct, ts(ch, 512)], in_=ps_j[ch][:],
                            func=AF.Copy)

                # ---- broadcast 1/s across partitions (recip done during J) ----
                bcast = small_pool.tile([128, HW_N], F32, tag="bcast")
                ps_b = bps_pool.tile([128, HW_N], F32, tag="bps")
                for ch in range(NCH):
                    nc.tensor.matmul(
                        ps_b[:, ts(ch, 512)], ones_row[:],
                        recip[:, ts(ch, 512)], start=True, stop=True)
                    nc.scalar.activation(
                        out=bcast[:, ts(ch, 512)], in_=ps_b[:, ts(ch, 512)],
                        func=AF.Copy)

                # ---- K: out = unnorm * bcast + cnn_proj (DVE, off PE path) ----
                for ct in range(MT):
                    out_t = out_pool.tile([128, HW_N], F32, tag="out_t")
                    for ch in range(NCH):
                        nc.vector.tensor_mul(
                            out=out_t[:, ts(ch, 512)],
                            in0=unnorm[:, ct, ts(ch, 512)],
                            in1=bcast[:, ts(ch, 512)])
                        nc.vector.tensor_add(
                            out=out_t[:, ts(ch, 512)],
                            in0=out_t[:, ts(ch, 512)],
                            in1=cnn_proj[:, ct, ts(ch, 512)])
                    nc.sync.dma_start(out=out_d[s, ct], in_=out_t[:])

    nc.compile()
    return nc


_CACHED_NC = None


def _get_nc():
    global _CACHED_NC
    if _CACHED_NC is None:
        _CACHED_NC = build_program()
    return _CACHED_NC


def make_in_maps(cnn_features, efficient_features, W_cnn, b_cnn, W_eff, b_eff,
                 W_q, b_q, W_k, b_k, W_v, b_v, gamma):
    f = np.float32
    bf = ml_dtypes.bfloat16
    g = np.asarray(gamma, f).reshape(-1)[0]
    W_qc = (np.asarray(W_q, f) @ np.asarray(W_cnn, f)).astype(f)
    b_qc = (np.asarray(W_q, f) @ np.asarray(b_cnn, f) + np.asarray(b_q, f))
    shared = {
        "wct": np.ascontiguousarray(np.asarray(W_cnn, f).T).reshape(KC_CNN, 128, HID),
        "wqct": np.ascontiguousarray(W_qc.T).reshape(KC_CNN, 128, HID),
        "wet": np.ascontiguousarray(np.asarray(W_eff, f).T).astype(bf).reshape(KC_EFF, 128, HID),
        "wkt": np.ascontiguousarray(np.asarray(W_k, f).T).astype(bf).reshape(MT, 128, HID),
        "wvt": np.ascontiguousarray(np.asarray(W_v, f).T * g).astype(bf).reshape(MT, 128, HID),
        "bcnn": np.ascontiguousarray(b_cnn, f).reshape(MT, 128, 1),
        "bqc": np.ascontiguousarray(b_qc, f).reshape(MT, 128, 1),
        "beff": np.ascontiguousarray(b_eff, f).reshape(MT, 128, 1),
        "bk": np.ascontiguousarray(b_k, f).reshape(MT, 128, 1),
        "bv": (np.ascontiguousarray(b_v, f) * g).reshape(1, HID),
    }
    xc = np.ascontiguousarray(cnn_features, f).reshape(B, KC_CNN, 128, HW_N)
    xe = np.ascontiguousarray(efficient_features, f).astype(bf).reshape(B, KC_EFF, 128, HW_N)
    in_maps = []
    for c in range(N_CORES):
        m = dict(shared)
        m["xc"] = np.ascontiguousarray(xc[c * S:(c + 1) * S])
        m["xe"] = np.ascontiguousarray(xe[c * S:(c + 1) * S])
        in_maps.append(m)
    return in_maps


def kernel(**inputs) -> np.ndarray:
    inputs = {k: np.asarray(v) for k, v in inputs.items()}
    nc = _get_nc()
    in_maps = make_in_maps(**inputs)
    res = run_bass_kernel_spmd(nc, in_maps, list(range(N_CORES)))
    out = np.concatenate([res.results[c]["out"] for c in range(N_CORES)], axis=0)
    return out.reshape(B, HID, 32, 32)


# revision 8
# speedup vs baseline: 1.1311x; 1.0982x over previous
"""CrossModalAttention Trainium2 kernel.

Per sample (C-major layouts, n = H*W = 1024 tokens):
    cnn_proj = W_cnn @ xc + b_cnn                [512, n]  f32 (residual path, exact)
    q  = (W_q @ W_cnn) @ xc + (W_q@b_cnn + b_q)  [512, n]  host-fused projection
    eff_proj = W_eff @ xe + b_eff                [512, n]
    k  = W_k @ eff_proj + b_k                    [512, n]
    vT = eff_proj^T @ (gamma*W_v)^T + gamma*b_v  [n, 512]  (v transposed, gamma folded)
    ST = k^T @ q                                 [n(keys m), n(queries)]
    ET = exp(ST)            (logits well-conditioned; no max-subtraction needed)
    s[n] = sum_m ET[m, n]   (ones-vector matmul accumulated in PSUM)
    out = (vT^T @ ET) * (1/s  broadcast) + cnn_proj

Sharding: data-parallel over batch, 4 samples per core on 8 cores.
Weight transposes / fusions are host-side; no on-chip transposes.
Projection GEMMs from DRAM inputs run as float32r (full PE rate);
the gamma-scaled attention path runs in bf16.
"""

import numpy as np
import ml_dtypes

import concourse.bacc as bacc
import concourse.bass as bass
import concourse.tile as tile
from concourse import mybir
from concourse.bass import ts
from concourse.bass_utils import run_bass_kernel_spmd

F32 = mybir.dt.float32
F32R = mybir.dt.float32r
BF16 = mybir.dt.bfloat16
AF = mybir.ActivationFunctionType

B, HW_N = 32, 1024
C_CNN, C_EFF, HID = 512, 1280, 512
N_CORES = 8
S = B // N_CORES          # samples per core
KC_CNN = C_CNN // 128     # 4
KC_EFF = C_EFF // 128     # 10
MT = HID // 128           # 4 output-channel tiles
NT = HW_N // 128          # 8 token tiles
NCH = HW_N // 512         # 2 free-dim chunks of 512


def build_program():
    nc = bacc.Bacc("TRN2", target_bir_lowering=False, debug=False,
                   num_devices=N_CORES)

    xc_d = nc.dram_tensor("xc", [S, KC_CNN, 128, HW_N], F32R, kind="ExternalInput")
    xe_d = nc.dram_tensor("xe", [S, KC_EFF, 128, HW_N], BF16, kind="ExternalInput")
    wct_d = nc.dram_tensor("wct", [KC_CNN, 128, HID], F32R, kind="ExternalInput")
    wqct_d = nc.dram_tensor("wqct", [KC_CNN, 128, HID], BF16, kind="ExternalInput")
    xcb_d = nc.dram_tensor("xcb", [S, KC_CNN, 128, HW_N], BF16, kind="ExternalInput")
    wet_d = nc.dram_tensor("wet", [KC_EFF, 128, HID], BF16, kind="ExternalInput")
    wkt_d = nc.dram_tensor("wkt", [MT, 128, HID], BF16, kind="ExternalInput")
    wvt_d = nc.dram_tensor("wvt", [MT, 128, HID], BF16, kind="ExternalInput")
    bcnn_d = nc.dram_tensor("bcnn", [MT, 128, 1], F32, kind="ExternalInput")
    bqc_d = nc.dram_tensor("bqc", [MT, 128, 1], F32, kind="ExternalInput")
    beff_d = nc.dram_tensor("beff", [MT, 128, 1], F32, kind="ExternalInput")
    bk_d = nc.dram_tensor("bk", [MT, 128, 1], F32, kind="ExternalInput")
    bv_d = nc.dram_tensor("bv", [1, HID], F32, kind="ExternalInput")
    out_d = nc.dram_tensor("out", [S, MT, 128, HW_N], F32, kind="ExternalOutput")

    with tile.TileContext(nc) as tc:
        with (
            tc.tile_pool(name="singles", bufs=1) as singles,
            tc.tile_pool(name="xs", bufs=6) as xs_pool,
            tc.tile_pool(name="proj", bufs=2) as proj_pool,
            tc.tile_pool(name="attn", bufs=1) as attn_pool,
            tc.tile_pool(name="small", bufs=2) as small_pool,
            tc.tile_pool(name="outp", bufs=2) as out_pool,
            tc.tile_pool(name="mmps", bufs=4, space="PSUM") as mmps,
            tc.tile_pool(name="sums", bufs=1, space="PSUM") as sums_pool,
            tc.tile_pool(name="bps", bufs=1, space="PSUM") as bps_pool,
        ):
            # ---- constants / weights (resident for the whole kernel) ----
            wcnn = singles.tile([128, KC_CNN, HID], F32R, tag="wcnn")
            wqc = singles.tile([128, KC_CNN, HID], BF16, tag="wqc")
            weff = singles.tile([128, KC_EFF, HID], BF16, tag="weff")
            wk = singles.tile([128, MT, HID], BF16, tag="wk")
            wv = singles.tile([128, MT, HID], BF16, tag="wv")
            for ko in range(KC_CNN):
                nc.sync.dma_start(out=wcnn[:, ko, :], in_=wct_d[ko])
            for ko in range(KC_CNN):
                nc.sync.dma_start(out=wqc[:, ko, :], in_=wqct_d[ko])
            for w_sb, w_dr, kn in ((weff, wet_d, KC_EFF), (wk, wkt_d, MT),
                                   (wv, wvt_d, MT)):
                for ko in range(kn):
                    nc.gpsimd.dma_start(out=w_sb[:, ko, :], in_=w_dr[ko])

            bcnn = singles.tile([128, MT], F32, tag="bcnn")
            bqc = singles.tile([128, MT], F32, tag="bqc")
            beff = singles.tile([128, MT], F32, tag="beff")
            bk = singles.tile([128, MT], F32, tag="bk")
            for b_sb, b_dr in ((bcnn, bcnn_d), (bqc, bqc_d), (beff, beff_d),
                               (bk, bk_d)):
                for ko in range(MT):
                    nc.gpsimd.dma_start(out=b_sb[:, ko:ko + 1], in_=b_dr[ko])

            bvt = singles.tile([128, HID], F32, tag="bvt")
            nc.gpsimd.dma_start(out=bvt[:], in_=bv_d[0].partition_broadcast(128))
            ones_bf = singles.tile([128, 1], BF16, tag="ones_bf")
            nc.vector.memset(ones_bf[:], 1.0)
            ones_row = singles.tile([1, 128], BF16, tag="ones_row")
            nc.vector.memset(ones_row[:], 1.0)

            def proj_sweep(dst, w_sb, bias, rhs_tiles, kcs, out_dtype_note=None):
                """dst[:, m, ch*512:] = sum_kc w_sb[:,kc,m*128:].T @ rhs(kc,ch) + bias[m]"""
                for ch in range(NCH):
                    ps = [mmps.tile([128, 512], F32, tag="mmps", name="ps")
                          for _ in range(MT)]
                    for kc in range(kcs):
                        rhs = rhs_tiles(kc, ch)
                        for m in range(MT):
                            nc.tensor.matmul(
                                ps[m][:], w_sb[:, kc, ts(m, 128)], rhs,
                                start=(kc == 0), stop=(kc == kcs - 1))
                    for m in range(MT):
                        nc.scalar.activation(
                            out=dst[:, m, ts(ch, 512)], in_=ps[m][:],
                            func=AF.Identity, bias=bias[:, m:m + 1], scale=1.0)

            for s in range(S):
                # ---- A: cnn_proj (f32, residual) ----
                cnn_proj = proj_pool.tile([128, MT, HW_N], F32, tag="cnn_proj")

                def rhs_xc(kc, ch, s=s):
                    xt = xs_pool.tile([128, 512], F32R, tag="xt", name="xt")
                    nc.sync.dma_start(out=xt[:], in_=xc_d[s, kc][:, ts(ch, 512)])
                    return xt[:]

                proj_sweep(cnn_proj, wcnn, bcnn, rhs_xc, KC_CNN)

                # ---- Q: q = Wqc @ xc + bqc (bf16, second xc sweep) ----
                q_sb = proj_pool.tile([128, MT, HW_N], BF16, tag="q", bufs=1)

                def rhs_xcb(kc, ch, s=s):
                    xt = xs_pool.tile([128, 512], BF16, tag="xt", name="xt")
                    nc.sync.dma_start(out=xt[:], in_=xcb_d[s, kc][:, ts(ch, 512)])
                    return xt[:]

                proj_sweep(q_sb, wqc, bqc, rhs_xcb, KC_CNN)

                # ---- B: eff_proj (bf16) ----
                eff_proj = proj_pool.tile([128, MT, HW_N], BF16, tag="eff_proj", bufs=1)

                def rhs_xe(kc, ch, s=s):
                    xt = xs_pool.tile([128, 512], BF16, tag="xt", name="xt")
                    nc.sync.dma_start(out=xt[:], in_=xe_d[s, kc][:, ts(ch, 512)])
                    return xt[:]

                proj_sweep(eff_proj, weff, beff, rhs_xe, KC_EFF)

                # ---- D: k = W_k @ eff_proj + b_k (bf16) ----
                k_sb = proj_pool.tile([128, MT, HW_N], BF16, tag="k", bufs=1)
                proj_sweep(k_sb, wk, bk,
                           lambda kc, ch: eff_proj[:, kc, ts(ch, 512)], MT)

                # ---- E: vT = eff_proj^T @ wv + bv (bf16) ----
                vT = attn_pool.tile([128, NT, HID], BF16, tag="vT")
                for mt in range(NT):
                    ps_e = mmps.tile([128, 512], F32, tag="mmps")
                    for kc in range(MT):
                        nc.tensor.matmul(
                            ps_e[:], eff_proj[:, kc, ts(mt, 128)], wv[:, kc, :],
                            start=(kc == 0), stop=(kc == MT - 1))
                    nc.vector.tensor_add(out=vT[:, mt, :], in0=ps_e[:],
                                         in1=bvt[:])

                # ---- F: ST = k^T @ q; ET = exp(ST); sums += 1^T ET ----
                eT = attn_pool.tile([128, NT, HW_N], BF16, tag="eT")
                sums_ps = sums_pool.tile([1, HW_N], F32, tag="sums")
                def emit_sums(mt):
                    for ch in range(NCH):
                        nc.tensor.matmul(
                            sums_ps[:, ts(ch, 512)], ones_bf[:],
                            eT[:, mt, ts(ch, 512)],
                            start=(mt == 0), stop=(mt == NT - 1))

                for mt in range(NT):
                    ps_f = [mmps.tile([128, 512], F32, tag="mmps", name="ps")
                            for _ in range(NCH)]
                    for kc in range(MT):
                        for ch in range(NCH):
                            nc.tensor.matmul(
                                ps_f[ch][:], k_sb[:, kc, ts(mt, 128)],
                                q_sb[:, kc, ts(ch, 512)],
                                start=(kc == 0), stop=(kc == MT - 1))
                    for ch in range(NCH):
                        nc.scalar.activation(
                            out=eT[:, mt, ts(ch, 512)], in_=ps_f[ch][:],
                            func=AF.Exp)
                    if mt > 0:
                        emit_sums(mt - 1)
                emit_sums(NT - 1)

                # ---- recip = 1 / sums (bf16; gamma already folded into wv) ----
                recip = small_pool.tile([1, HW_N], BF16, tag="recip")
                with nc.allow_low_precision(
                        reason="softmax denominators are gamma-scaled; "
                               "bf16 suffices"):
                    nc.vector.reciprocal(out=recip[:], in_=sums_ps[:])

                # ---- J: unnorm = vT^T @ ET (ACT-drained); bcast + K trail ----
                unnorm = attn_pool.tile([128, MT, HW_N], F32, tag="unnorm")
                bcast = small_pool.tile([128, HW_N], F32, tag="bcast")

                def emit_k(ct):
                    out_t = out_pool.tile([128, HW_N], F32, tag="out_t",
                                          name="out_t")
                    for ch in range(NCH):
                        nc.vector.tensor_mul(
                            out=out_t[:, ts(ch, 512)],
                            in0=unnorm[:, ct, ts(ch, 512)],
                            in1=bcast[:, ts(ch, 512)])
                        nc.vector.tensor_add(
                            out=out_t[:, ts(ch, 512)],
                            in0=out_t[:, ts(ch, 512)],
                            in1=cnn_proj[:, ct, ts(ch, 512)])
                    nc.sync.dma_start(out=out_d[s, ct], in_=out_t[:])

                for ct in range(MT):
                    ps_j = [mmps.tile([128, 512], F32, tag="mmps", name="ps")
                            for _ in range(NCH)]
                    for mt in range(NT):
                        for ch in range(NCH):
                            nc.tensor.matmul(
                                ps_j[ch][:], vT[:, mt, ts(ct, 128)],
                                eT[:, mt, ts(ch, 512)],
                                start=(mt == 0), stop=(mt == NT - 1))
                    for ch in range(NCH):
                        nc.scalar.activation(
                            out=unnorm[:, ct, ts(ch, 512)], in_=ps_j[ch][:],
                            func=AF.Copy)
                    if ct == 1:
                        # broadcast 1/s across partitions (recip ready by now)
                        ps_b = bps_pool.tile([128, HW_N], F32, tag="bps")
                        for ch in range(NCH):
                            nc.tensor.matmul(
                                ps_b[:, ts(ch, 512)], ones_row[:],
                                recip[:, ts(ch, 512)], start=True, stop=True)
                            nc.scalar.activation(
                                out=bcast[:, ts(ch, 512)],
                                in_=ps_b[:, ts(ch, 512)], func=AF.Copy)
                    if ct >= 2:
                        emit_k(ct - 2)
                emit_k(MT - 2)
                emit_k(MT - 1)

    nc.compile()
    return nc


_CACHED_NC = None


def _get_nc():
    global _CACHED_NC
    if _CACHED_NC is None:
        _CACHED_NC = build_program()
    return _CACHED_NC


def make_in_maps(cnn_features, efficient_features, W_cnn, b_cnn, W_eff, b_eff,
                 W_q, b_q, W_k, b_k, W_v, b_v, gamma):
    f = np.float32
    bf = ml_dtypes.bfloat16
    g = np.asarray(gamma, f).reshape(-1)[0]
    W_qc = (np.asarray(W_q, f) @ np.asarray(W_cnn, f)).astype(f)
    b_qc = (np.asarray(W_q, f) @ np.asarray(b_cnn, f) + np.asarray(b_q, f))
    shared = {
        "wct": np.ascontiguousarray(np.asarray(W_cnn, f).T).reshape(KC_CNN, 128, HID),
        "wqct": np.ascontiguousarray(W_qc.T).astype(bf).reshape(KC_CNN, 128, HID),
        "wet": np.ascontiguousarray(np.asarray(W_eff, f).T).astype(bf).reshape(KC_EFF, 128, HID),
        "wkt": np.ascontiguousarray(np.asarray(W_k, f).T).astype(bf).reshape(MT, 128, HID),
        "wvt": np.ascontiguousarray(np.asarray(W_v, f).T * g).astype(bf).reshape(MT, 128, HID),
        "bcnn": np.ascontiguousarray(b_cnn, f).reshape(MT, 128, 1),
        "bqc": np.ascontiguousarray(b_qc, f).reshape(MT, 128, 1),
        "beff": np.ascontiguousarray(b_eff, f).reshape(MT, 128, 1),
        "bk": np.ascontiguousarray(b_k, f).reshape(MT, 128, 1),
        "bv": (np.ascontiguousarray(b_v, f) * g).reshape(1, HID),
    }
    xc = np.ascontiguousarray(cnn_features, f).reshape(B, KC_CNN, 128, HW_N)
    xe = np.ascontiguousarray(efficient_features, f).astype(bf).reshape(B, KC_EFF, 128, HW_N)
    in_maps = []
    for c in range(N_CORES):
        m = dict(shared)
        m["xc"] = np.ascontiguousarray(xc[c * S:(c + 1) * S])
        m["xcb"] = m["xc"].astype(bf)
        m["xe"] = np.ascontiguousarray(xe[c * S:(c + 1) * S])
        in_maps.append(m)
    return in_maps


def kernel(**inputs) -> np.ndarray:
    inputs = {k: np.asarray(v) for k, v in inputs.items()}
    nc = _get_nc()
    in_maps = make_in_maps(**inputs)
    res = run_bass_kernel_spmd(nc, in_maps, list(range(N_CORES)))
    out = np.concatenate([res.results[c]["out"] for c in range(N_CORES)], axis=0)
    return out.reshape(B, HID, 32, 32)
